# revision 1
# baseline (speedup 1.0000x reference)
"""Distributed Trainium2 attention kernel (8 NeuronCores, head tensor-parallel).

Reference semantics (T=4096, D=2048, H=16, DH=128):
  qkv = bf16(x @ W_qkv); q,k,v per head; RoPE(split-half) on q,k;
  mask = ((m_q & m_k) | eye) & causal; softmax(q k^T / sqrt(DH) masked);
  out = bf16((probs @ v) @ W_out)

Sharding: head tensor-parallel. Core c owns heads (2c, 2c+1): W_qkv column
shard, W_out row shard, full x (replicated, passed pre-transposed).
Each core computes its heads' SDPA, its out-projection partial, then a
chunked ReduceScatter sums partials; core c emits output rows
[chunk*1024 + c*128 : +128) for each of the 4 chunks. Host reassembles.

Device-side layout choices:
  - x passed as xT [D, T] so the D contraction dim is the partition dim.
  - q,k computed weight-stationary -> born transposed [DH, T]; v
    transposed back to natural [T, DH] via PE (PV lhsT layout).
  - RoPE: partition-rotate by 64 via a permutation-matrix matmul on PE,
    sign folded into a host-precomputed ssinT table; combine on DVE.
  - SDPA in transposed-scores form: scoresT[k, q] tiles over 512-query
    quads; exp (no max-subtraction; scores are O(5) here) evacuates the
    scores psum straight into the PV rhs -- no probs transposes.
  - key padding mask folded into the exp bias (per-k = per-partition);
    within-block causal via one precomputed 0/1 [128,128] multiply.
  - softmax denominators via a ones-column matmul, transposed to
    q-partition layout with 4 tiny K=1 matmuls; normalization deferred
    to the out-projection epilogue (per-partition scalars there), with
    a mid-accumulation ratio scale to handle the two heads' different
    denominators in one psum group.
  - masked queries (attend only self) fixed by blending v^T via colmask
    zeroing + (1-m) add; all blend scalars stay per-partition.
"""

import os
import sys

import numpy as np

sys.path.insert(0, "/opt/trn_rl_repo")

import ml_dtypes

BF16 = ml_dtypes.bfloat16

# problem constants (hardcoded per harness contract)
T, D, H, DH = 4096, 2048, 16, 128
N_CORES = 8
ROPE_BASE = 10000.0


def _rs_chunk_sizes(qb_n, rs_chunks):
    """Reduce-scatter chunk sizes in q-blocks: front-loaded so the final
    collective (pure exposed tail) is tiny."""
    if qb_n == 32:
        return [6, 6, 6, 5, 4, 2, 2, 1]
    per = qb_n // rs_chunks
    return [per] * rs_chunks


def build_nc(
    t=T,
    d=D,
    n_cores=N_CORES,
    hl=H // N_CORES,  # heads per core
    kch=512,  # scores k-chunk (free dim of scores matmul)
    tch=512,  # qkv t-chunk
    rs_chunks=8,  # reduce-scatter chunks
):
    import concourse.bass as bass
    import concourse.mybir as mybir
    import concourse.tile as tile
    from concourse import bacc
    from concourse.masks import make_identity

    f32 = mybir.dt.float32
    bf16 = mybir.dt.bfloat16

    P = 128
    kd = d // P  # contraction chunks for qkv
    qb_n = t // P  # q-blocks of 128 rows
    nt = t // tch  # t-chunks in qkv phase
    jl = hl * P  # local out-proj contraction width
    chunk_sizes = _rs_chunk_sizes(qb_n, rs_chunks)
    chunk_starts = [0]
    for cs_ in chunk_sizes:
        chunk_starts.append(chunk_starts[-1] + cs_)
    qb_to_chunk = {}
    for ci_, cs_ in enumerate(chunk_sizes):
        for ri_ in range(cs_):
            qb_to_chunk[chunk_starts[ci_] + ri_] = (ci_, ri_)
    t_out = t // n_cores  # output rows per core
    scale = 1.0 / np.sqrt(DH)

    nc = bacc.Bacc(
        "TRN2", target_bir_lowering=False, debug=False, num_devices=n_cores
    )

    xT = nc.dram_tensor("xT", [d, t], bf16, kind="ExternalInput").ap()
    wqkv = nc.dram_tensor("wqkv", [d, 3 * jl], bf16, kind="ExternalInput").ap()
    wout_d = nc.dram_tensor("wout", [jl, d], bf16, kind="ExternalInput").ap()
    cosT_d = nc.dram_tensor("cosT", [P, t], f32, kind="ExternalInput").ap()
    ssinT_d = nc.dram_tensor("ssinT", [P, t], f32, kind="ExternalInput").ap()
    # colmask: mask[k] as 0/1 bf16, pre-broadcast to all 128 partitions
    colmask_d = nc.dram_tensor("colmask", [P, t], bf16, kind="ExternalInput").ap()
    # rqT[p, qb] = 0 if mask[qb*128+p] else -1e9 (folded into exp bias)
    rqT_d = nc.dram_tensor("rqT", [P, qb_n], f32, kind="ExternalInput").ap()
    # dvalB[p, q] = 1 - mask[q], broadcast to all partitions
    dvalB_d = nc.dram_tensor("dvalB", [P, t], bf16, kind="ExternalInput").ap()
    # dvalT[p, qb] = 1 - mask[qb*128+p] (q-partition layout)
    dvalT_d = nc.dram_tensor("dvalT", [P, qb_n], f32, kind="ExternalInput").ap()
    # mqT[p, qb] = mask[qb*128+p] (q-partition layout)
    mqT_d = nc.dram_tensor("mqT", [P, qb_n], f32, kind="ExternalInput").ap()
    # cmask128[p, j] = 1 if j >= p else 0 (within-block causal, T-orientation)
    cmask128_d = nc.dram_tensor("cmask128", [P, P], bf16, kind="ExternalInput").ap()
    out_d = nc.dram_tensor("out", [t_out, d], bf16, kind="ExternalOutput").ap()

    with tile.TileContext(nc) as tc:
        with tc.tile_pool(name="persist", bufs=1) as persist:
            # persistent SBUF tensors
            ident = persist.tile([P, P], bf16, name="ident")
            make_identity(nc, ident)
            wq_sb = persist.tile([P, kd, 3 * hl, P], bf16, name="wq_sb")
            wqkv_r = wqkv.rearrange("(kd p) (c j) -> p kd c j", p=P, j=P)
            for k in range(kd):
                nc.sync.dma_start(wq_sb[:, k], wqkv_r[:, k])
            wout_sb = persist.tile([P, hl, d], bf16, name="wout_sb")
            nc.sync.dma_start(wout_sb, wout_d.rearrange("(h p) x -> p h x", p=P))

            # per-head persistent activations
            qT = [persist.tile([P, t], bf16, name=f"qT{h}") for h in range(hl)]
            kT = [persist.tile([P, t], bf16, name=f"kT{h}") for h in range(hl)]
            vT = [persist.tile([P, t], bf16, name=f"vT{h}") for h in range(hl)]
            v_nat = [
                persist.tile([P, qb_n, P], bf16, name=f"vnat{h}") for h in range(hl)
            ]
            oT = [persist.tile([P, t], bf16, name=f"oT{h}") for h in range(hl)]

            # ---------------- phase 1: qkv + rope + v transpose ----------
            with (
                tc.tile_pool(name="ph1", bufs=2) as ph1,
                tc.tile_pool(name="cs", bufs=1) as cspool,
                tc.tile_pool(name="ps_qkv", bufs=1, space="PSUM") as ps_qkv,
                tc.tile_pool(name="ps_aux", bufs=2, space="PSUM") as ps_aux,
            ):
                cosT_sb = cspool.tile([P, t], f32, name="cosT_sb")
                nc.sync.dma_start(cosT_sb, cosT_d)
                ssinT_sb = cspool.tile([P, t], f32, name="ssinT_sb")
                nc.sync.dma_start(ssinT_sb, ssinT_d)

                for tc_i in range(nt):
                    tsl = slice(tc_i * tch, (tc_i + 1) * tch)
                    xt = ph1.tile([P, kd, tch], bf16, tag="xt")
                    xT_r = xT.rearrange("(kd p) x -> p kd x", p=P)
                    for k in range(kd):
                        nc.sync.dma_start(xt[:, k], xT_r[:, k, tsl])
                    for c in range(3 * hl):  # q0,q1,k0,k1,v0,v1
                        ps = ps_qkv.tile([P, tch], mybir.dt.float32, tag=f"ps{c}")
                        for k in range(kd):
                            nc.tensor.matmul(
                                ps,
                                lhsT=wq_sb[:, k, c],
                                rhs=xt[:, k],
                                start=(k == 0),
                                stop=(k == kd - 1),
                            )
                        if c < 2 * hl:  # q or k: cast, rotate, rope-combine
                            dst = qT[c] if c < hl else kT[c - hl]
                            qbf = ph1.tile([P, tch], bf16, tag="qbf")
                            nc.scalar.copy(qbf, ps)
                            # rotate-half: partition shift by 64 via two
                            # SBUF->SBUF DMAs (keeps PE free)
                            shift = ph1.tile([P, tch], bf16, tag="shift")
                            nc.sync.dma_start(shift[0:64], qbf[64:128])
                            nc.sync.dma_start(shift[64:128], qbf[0:64])
                            t1 = ph1.tile([P, tch], f32, tag="t1")
                            nc.vector.tensor_tensor(
                                t1, qbf, cosT_sb[:, tsl], mybir.AluOpType.mult
                            )
                            t2 = ph1.tile([P, tch], f32, tag="t2")
                            nc.vector.tensor_tensor(
                                t2, shift, ssinT_sb[:, tsl], mybir.AluOpType.mult
                            )
                            nc.vector.tensor_tensor(
                                dst[:, tsl], t1, t2, mybir.AluOpType.add
                            )
                        else:  # v: just cast
                            nc.scalar.copy(vT[c - 2 * hl][:, tsl], ps)

                # v: [DH, T] -> natural [T-block, DH] tiles
                for h in range(hl):
                    for b in range(qb_n):
                        pst = ps_aux.tile([P, P], bf16, tag="aux")
                        nc.tensor.transpose(
                            pst, vT[h][:, b * P : (b + 1) * P], ident
                        )
                        nc.scalar.copy(v_nat[h][:, b], pst)

            # ---------------- phase 2: SDPA + out-proj + RS --------------
            # Transposed-scores formulation: scoresT[k, q] tiles per 128-k
            # block over a 512-query "quad"; exp evacuates psum straight to
            # the PV rhs; denominator via a ones-column matmul; softmax
            # normalization + masked-row fixup fused into the single oT
            # evacuation (per-query scalars partition-broadcast on gpsimd).
            qw = 512  # queries per quad
            with (
                tc.tile_pool(name="ph2", bufs=3) as ph2,
                tc.tile_pool(name="pt", bufs=3) as ptpool,
                tc.tile_pool(name="msk", bufs=1) as mskpool,
                tc.tile_pool(name="dram", bufs=1, space="DRAM") as dram,
                tc.tile_pool(name="ps_s", bufs=2, space="PSUM") as ps_s,
                tc.tile_pool(name="ps_o", bufs=1, space="PSUM") as ps_o,
                tc.tile_pool(name="ps_d", bufs=2, space="PSUM") as ps_d,
                tc.tile_pool(name="ps_dt", bufs=1, space="PSUM") as ps_dt,
                tc.tile_pool(name="ps_out", bufs=2, space="PSUM") as ps_out,
            ):
                colmask_sb = mskpool.tile([P, t], bf16, name="colmask_sb")
                nc.sync.dma_start(colmask_sb, colmask_d)
                rqT_sb = mskpool.tile([P, qb_n], f32, name="rqT_sb")
                nc.sync.dma_start(rqT_sb, rqT_d)
                dvalB_sb = mskpool.tile([P, t], bf16, name="dvalB_sb")
                nc.sync.dma_start(dvalB_sb, dvalB_d)
                cm128_sb = mskpool.tile([P, P], bf16, name="cm128_sb")
                nc.sync.dma_start(cm128_sb, cmask128_d)
                ones_sb = mskpool.tile([P, 1], bf16, name="ones_sb")
                nc.vector.memset(ones_sb, 1.0)
                dvalT_sb = mskpool.tile([P, qb_n], f32, name="dvalT_sb")
                nc.sync.dma_start(dvalT_sb, dvalT_d)
                mqT_sb = mskpool.tile([P, qb_n], f32, name="mqT_sb")
                nc.sync.dma_start(mqT_sb, mqT_d)
                rs_in = [
                    dram.tile([cs_ * P, d], bf16, name=f"rs_in{ci}")
                    for ci, cs_ in enumerate(chunk_sizes)
                ]
                rs_out = [
                    dram.tile([cs_ * P // n_cores, d], bf16, name=f"rs_out{ci}")
                    for ci, cs_ in enumerate(chunk_sizes)
                ]

                n_quads = t // qw
                qb_per_quad = qw // P  # 4
                for g in range(n_quads):
                    gsl = slice(g * qw, (g + 1) * qw)
                    nsk = (g + 1) * qb_per_quad  # causal k-blocks for quad
                    dsum_bfs = []
                    invs = {}
                    for h in range(hl):
                        pso = ps_o.tile([P, qw], f32, tag="oT")
                        psd = ps_d.tile([1, qw], f32, tag="den")

                        def emit_score(sk, h=h, g=g):
                            br = sk - g * qb_per_quad  # >=0 in diag region
                            lo = br * P if br >= 0 else 0
                            psT = ps_s.tile([P, qw], f32, tag="scT",
                                            name="psT")
                            nc.tensor.matmul(
                                psT[:, lo:],
                                lhsT=kT[h][:, sk * P : (sk + 1) * P],
                                rhs=qT[h][:, g * qw + lo : (g + 1) * qw],
                                start=True,
                                stop=True,
                            )
                            pT = ptpool.tile([P, qw], bf16, tag="pT",
                                             name="pT")
                            # exp; per-k padding mask folded into the bias
                            nc.scalar.activation(
                                pT[:, lo:],
                                psT[:, lo:],
                                mybir.ActivationFunctionType.Exp,
                                scale=float(scale),
                                bias=rqT_sb[:, sk : sk + 1],
                            )
                            if br >= 0:
                                # within-block causal on the partial 128 cols
                                nc.vector.tensor_tensor(
                                    pT[:, lo : lo + P],
                                    pT[:, lo : lo + P],
                                    cm128_sb,
                                    mybir.AluOpType.mult,
                                )
                            return pT, lo

                        def emit_pv(sk, pT, lo, h=h, pso=pso, psd=psd,
                                    nsk=nsk):
                            nc.tensor.matmul(
                                pso[:, lo:],
                                lhsT=v_nat[h][:, sk],
                                rhs=pT[:, lo:],
                                start=(sk == 0),
                                stop=(sk == nsk - 1),
                            )
                            nc.tensor.matmul(
                                psd[:, lo:],
                                lhsT=ones_sb,
                                rhs=pT[:, lo:],
                                start=(sk == 0),
                                stop=(sk == nsk - 1),
                            )

                        # software-pipelined emission (lookahead 2) so the
                        # PE stream never stalls on exp: scT(sk+1), scT(sk+2)
                        # run while exp(sk) finishes, then PV(sk)
                        LA = 2
                        stage = {}
                        for sk in range(nsk):
                            stage[sk] = emit_score(sk)
                            if sk - LA >= 0:
                                emit_pv(sk - LA, *stage.pop(sk - LA))
                        for sk in range(max(0, nsk - LA), nsk):
                            emit_pv(sk, *stage.pop(sk))
                        # oT kept UNNORMALIZED (bf16 is scale-free); masked-q
                        # garbage zeroed via colmask; masked queries attend
                        # only themselves -> blend in v^T * (1-m[q])
                        nc.vector.tensor_tensor(
                            oT[h][:, gsl], pso, colmask_sb[:, gsl],
                            mybir.AluOpType.mult,
                        )
                        vblend = ph2.tile([P, qw], bf16, tag="vblend")
                        nc.gpsimd.tensor_tensor(
                            vblend, vT[h][:, gsl], dvalB_sb[:, gsl],
                            mybir.AluOpType.mult,
                        )
                        nc.vector.tensor_tensor(
                            oT[h][:, gsl], oT[h][:, gsl], vblend,
                            mybir.AluOpType.add,
                        )
                        dsum_bf = ph2.tile([1, qw], bf16, tag=f"dsum{h}")
                        nc.vector.tensor_copy(dsum_bf, psd)
                        dsum_bfs.append(dsum_bf)

                    # denominators -> q-partition layout (both heads, off the
                    # critical PE path): transpose each [1, 512] row into
                    # [128, 4] via 4 tiny K=1 matmuls so the reciprocal runs
                    # on all 128 lanes; inv3 = m[q]/denom + (1-m[q])
                    gq = slice(g * qb_per_quad, (g + 1) * qb_per_quad)
                    for h in range(hl):
                        denT = ps_dt.tile([P, qb_per_quad], f32, tag="denT")
                        for j in range(qb_per_quad):
                            nc.tensor.matmul(
                                denT[:, j : j + 1],
                                lhsT=dsum_bfs[h][0:1, j * P : (j + 1) * P],
                                rhs=ones_sb[0:1, 0:1],
                                start=True,
                                stop=True,
                            )
                        rec = ph2.tile([P, qb_per_quad], f32, tag=f"rec{h}")
                        nc.vector.reciprocal(rec, denT)
                        inv3 = ph2.tile([P, qb_per_quad], f32, tag=f"inv3{h}")
                        nc.vector.tensor_tensor(
                            inv3, rec, mqT_sb[:, gq], mybir.AluOpType.mult
                        )
                        nc.vector.tensor_tensor(
                            inv3, inv3, dvalT_sb[:, gq], mybir.AluOpType.add
                        )
                        invs[h] = inv3
                        if h == hl - 1 and hl > 1:
                            # jnv = m*denom + (1-m); r = inv3_0 * jnv_1 so a
                            # mid-accumulation psum scale by r followed by a
                            # final scale by inv3_1 yields per-head softmax
                            # normalization inside one accumulation group
                            jnv = ph2.tile([P, qb_per_quad], f32, tag="jnv")
                            nc.vector.tensor_tensor(
                                jnv, denT, mqT_sb[:, gq], mybir.AluOpType.mult
                            )
                            nc.vector.tensor_tensor(
                                jnv, jnv, dvalT_sb[:, gq], mybir.AluOpType.add
                            )
                            rsc = ph2.tile([P, qb_per_quad], f32, tag="rsc")
                            nc.vector.tensor_tensor(
                                rsc, invs[0], jnv, mybir.AluOpType.mult
                            )
                    inv3 = invs[hl - 1]

                    # out-projection + RS for the quad's 4 q-blocks,
                    # software-pipelined (lookahead 1) so the h1 matmul of
                    # step n overlaps the DVE scales of step n-1
                    ntiles = d // 512

                    def op_flush(qb2, partial2):
                        ci, ri = qb_to_chunk[qb2]
                        nc.sync.dma_start(
                            rs_in[ci][ri * P : (ri + 1) * P, :], partial2
                        )
                        if ri == chunk_sizes[ci] - 1:
                            nc.gpsimd.collective_compute(
                                "ReduceScatter",
                                mybir.AluOpType.add,
                                replica_groups=[list(range(n_cores))],
                                ins=[rs_in[ci].opt()],
                                outs=[rs_out[ci].opt()],
                            )
                            rows = chunk_sizes[ci] * P // n_cores
                            orow = chunk_starts[ci] * P // n_cores
                            nc.sync.dma_start(
                                out_d[orow : orow + rows, :],
                                rs_out[ci],
                            )

                    def op_fin(pso2, qsl2, nsl2, partial2, qq2, qb2):
                        if hl > 1:
                            nc.tensor.matmul(
                                pso2,
                                lhsT=oT[1][:, qsl2],
                                rhs=wout_sb[:, 1, nsl2],
                                start=False,
                                stop=True,
                                skip_group_check=True,
                            )
                        nc.vector.tensor_scalar_mul(
                            partial2[:, nsl2], pso2, inv3[:, qq2 : qq2 + 1]
                        )
                        if nsl2.stop == d:  # last ntile -> partial complete
                            op_flush(qb2, partial2)

                    prev = None
                    for qq in range(qb_per_quad):
                        qb = g * qb_per_quad + qq
                        qsl = slice(qb * P, (qb + 1) * P)
                        partial = ph2.tile([P, d], bf16, tag="partial",
                                           name="partial")
                        for ntile in range(ntiles):
                            nsl = slice(ntile * 512, (ntile + 1) * 512)
                            pso2 = ps_out.tile([P, 512], f32, tag="outp",
                                               name="pso2")
                            nc.tensor.matmul(
                                pso2,
                                lhsT=oT[0][:, qsl],
                                rhs=wout_sb[:, 0, nsl],
                                start=True,
                                stop=True,
                            )
                            if hl > 1:
                                nc.vector.tensor_scalar_mul(
                                    pso2, pso2, rsc[:, qq : qq + 1]
                                )
                            if prev is not None:
                                op_fin(*prev)
                            prev = (pso2, qsl, nsl, partial, qq, qb)
                    op_fin(*prev)

    nc.compile()
    return nc


def prepare_in_maps(x, W_qkv, W_out, cos, sin, mask, n_cores=N_CORES, hl=H // N_CORES):
    """Host-side sharding. Returns list of per-core input dicts."""
    t, d = x.shape
    x = np.asarray(x, dtype=BF16)
    W_qkv = np.asarray(W_qkv, dtype=BF16)
    W_out = np.asarray(W_out, dtype=BF16)
    cos = np.asarray(cos, dtype=np.float32)
    sin = np.asarray(sin, dtype=np.float32)
    m = np.asarray(mask, dtype=bool)

    xT = np.ascontiguousarray(x.T)
    cosT = np.ascontiguousarray(cos.T)
    sign = np.where(np.arange(DH) < DH // 2, -1.0, 1.0).astype(np.float32)
    ssinT = np.ascontiguousarray(sin.T * sign[:, None])

    mf = m.astype(np.float32)
    colmask = np.ascontiguousarray(
        np.broadcast_to(mf.astype(BF16)[None, :], (DH, t))
    )
    rqT = np.ascontiguousarray(
        np.where(mf, np.float32(0.0), np.float32(-1e9)).reshape(-1, DH).T
    )
    dvalB = np.ascontiguousarray(
        np.broadcast_to((1.0 - mf).astype(BF16)[None, :], (DH, t))
    )
    dvalT = np.ascontiguousarray((1.0 - mf).astype(np.float32).reshape(-1, DH).T)
    mqT = np.ascontiguousarray(mf.astype(np.float32).reshape(-1, DH).T)
    cmask128 = (np.arange(DH)[None, :] >= np.arange(DH)[:, None]).astype(BF16)

    n_heads = W_qkv.shape[1] // 3 // DH
    in_maps = []
    for c in range(n_cores):
        hs = [c * hl + i for i in range(hl)]
        cols = [W_qkv[:, (s * n_heads + h) * DH : (s * n_heads + h) * DH + DH]
                for s in range(3) for h in hs]
        wqkv_c = np.ascontiguousarray(np.concatenate(cols, axis=1))
        wout_c = np.ascontiguousarray(
            W_out[hs[0] * DH : (hs[-1] + 1) * DH, :]
        )
        in_maps.append(
            {
                "xT": xT,
                "wqkv": wqkv_c,
                "wout": wout_c,
                "cosT": cosT,
                "ssinT": ssinT,
                "colmask": colmask,
                "rqT": rqT,
                "dvalB": dvalB,
                "dvalT": dvalT,
                "mqT": mqT,
                "cmask128": cmask128,
            }
        )
    return in_maps


_CACHED_NC = None


def assemble(results, t=T, d=D, n_cores=N_CORES, rs_chunks=8):
    """Reassemble per-core ReduceScatter slices into the full output."""
    P = 128
    qb_n = t // P
    chunk_sizes = _rs_chunk_sizes(qb_n, rs_chunks)
    out = np.empty((t, d), dtype=BF16)
    for c in range(n_cores):
        oc = np.asarray(results[c]["out"])
        if oc.dtype != BF16:
            oc = oc.view(BF16)
        row0 = 0  # chunk start in global rows
        orow = 0  # chunk start in per-core output rows
        for cs_ in chunk_sizes:
            rows = cs_ * P // n_cores
            lo = row0 + c * rows
            out[lo : lo + rows] = oc[orow : orow + rows]
            row0 += cs_ * P
            orow += rows
    return out


def kernel(x, W_qkv, W_out, cos, sin, mask):
    """Full inputs in, full output out. Shards across 8 NeuronCores."""
    global _CACHED_NC
    from concourse import bass_utils

    if _CACHED_NC is None:
        _CACHED_NC = build_nc()
    nc = _CACHED_NC

    in_maps = prepare_in_maps(x, W_qkv, W_out, cos, sin, mask)
    res = bass_utils.run_bass_kernel_spmd(
        nc, in_maps, core_ids=list(range(N_CORES))
    )
    return assemble(res.results)



# revision 9
# speedup vs baseline: 1.2443x; 1.2443x over previous
"""Distributed Trainium2 attention kernel (8 NeuronCores, head tensor-parallel).

Reference semantics (T=4096, D=2048, H=16, DH=128):
  qkv = bf16(x @ W_qkv); q,k,v per head; RoPE(split-half) on q,k;
  mask = ((m_q & m_k) | eye) & causal; softmax(q k^T / sqrt(DH) masked);
  out = bf16((probs @ v) @ W_out)

Sharding: head tensor-parallel for qkv+SDPA (core c owns heads 2c, 2c+1),
then an AllToAll redistributes the small per-head attention outputs o so
that each core owns 64 output ROWS per 512-query quad (512 rows total)
and computes the full out-projection locally against a replicated W_out.
This moves 8x fewer bytes than reduce-scattering output partials.

Device-side layout choices:
  - x passed as xT [D, T] so the D contraction dim is the partition dim.
  - q,k computed weight-stationary -> born transposed [DH, T]; v
    transposed back to natural [T, DH] via PE (PV lhsT layout).
  - RoPE: partition-rotate by 64 via two SBUF->SBUF DMAs, sign folded
    into a host-precomputed ssinT table; combine on DVE.
  - SDPA in transposed-scores form: scoresT[k, q] tiles over 512-query
    quads; exp (no max-subtraction; scores are O(5)) evacuates the
    scores psum straight into the PV rhs; key padding mask folded into
    the exp bias (per-k = per-partition); within-block causal via one
    precomputed 0/1 [128,128] multiply; softmax denominators via a
    ones-column matmul.
  - Normalization fused into the oT evacuation: inv = m_q/(den+(1-m_q))
    computed on one partition, broadcast to all 128 partitions with a
    K=1 ones-outer-product matmul on PE, then oT = pso*bc + vT*(1-m_q)
    (vT premultiplied by (1-m_q) once). Masked queries thereby attend
    only to themselves; out-projection needs no per-tile scaling.
  - bc matmuls and evacuations lag one head-sequence behind the PE
    block stream so the PE never waits on the DVE denominator chain.
  - A2A per quad: [2048, 64] bf16 (shard j = my heads' oT columns for
    rank j's 64 rows of this quad). Collective-dependent loads ride the
    GpSimd DMA queue so they cannot head-of-line-block the Sync queue.
"""

import os
import sys

import numpy as np

sys.path.insert(0, "/opt/trn_rl_repo")

import ml_dtypes

BF16 = ml_dtypes.bfloat16

# problem constants (hardcoded per harness contract)
T, D, H, DH = 4096, 2048, 16, 128
N_CORES = 8
ROPE_BASE = 10000.0


def build_nc(
    t=T,
    d=D,
    n_cores=N_CORES,
    hl=H // N_CORES,  # heads per core
    tch=512,  # qkv t-chunk
):
    import concourse.bass as bass
    import concourse.mybir as mybir
    import concourse.tile as tile
    from concourse import bacc
    from concourse.masks import make_identity

    f32 = mybir.dt.float32
    bf16 = mybir.dt.bfloat16

    P = 128
    kd = d // P  # contraction chunks for qkv
    qb_n = t // P  # q-blocks of 128 rows
    nt = t // tch  # t-chunks in qkv phase
    qw = 512  # queries per quad
    n_quads = t // qw
    qb_per_quad = qw // P  # 4
    rows_per_rank = qw // n_cores  # 64 rows each rank owns per quad
    n_pairs = n_quads // 2
    t_out = t // n_cores  # output rows per core
    scale = 1.0 / np.sqrt(DH)

    nc = bacc.Bacc(
        "TRN2", target_bir_lowering=False, debug=False, num_devices=n_cores
    )

    xT = nc.dram_tensor("xT", [d, t], bf16, kind="ExternalInput").ap()
    wqkv = nc.dram_tensor("wqkv", [d, 3 * hl * P], bf16, kind="ExternalInput").ap()
    wout_d = nc.dram_tensor("wout", [d, d], bf16, kind="ExternalInput").ap()
    cosT_d = nc.dram_tensor("cosT", [P, t], f32, kind="ExternalInput").ap()
    ssinT_d = nc.dram_tensor("ssinT", [P, t], f32, kind="ExternalInput").ap()
    # rqT[p, qb] = 0 if mask[qb*128+p] else -1e9 (folded into exp bias)
    rqT_d = nc.dram_tensor("rqT", [P, qb_n], f32, kind="ExternalInput").ap()
    # dvalB[p, q] = 1 - mask[q], broadcast to all partitions
    dvalB_d = nc.dram_tensor("dvalB", [P, t], bf16, kind="ExternalInput").ap()
    # mrow[0, q] = mask[q] ; dvalrow[0, q] = 1 - mask[q]
    mrow_d = nc.dram_tensor("mrow", [1, t], bf16, kind="ExternalInput").ap()
    dvalrow_d = nc.dram_tensor("dvalrow", [1, t], f32, kind="ExternalInput").ap()
    # cmask128[p, j] = 1 if j >= p else 0 (within-block causal, T-orientation)
    cmask128_d = nc.dram_tensor("cmask128", [P, P], bf16, kind="ExternalInput").ap()
    out_d = nc.dram_tensor("out", [t_out, d], bf16, kind="ExternalOutput").ap()

    with tile.TileContext(nc) as tc:
        with tc.tile_pool(name="persist", bufs=1) as persist:
            ident = persist.tile([P, P], bf16, name="ident")
            make_identity(nc, ident)
            ones_col = persist.tile([P, 1], bf16, name="ones_col")
            nc.vector.memset(ones_col, 1.0)
            ones_row = persist.tile([1, P], bf16, name="ones_row")
            nc.vector.memset(ones_row, 1.0)
            rqT_sb = persist.tile([P, qb_n], f32, name="rqT_sb")
            cm128_sb = persist.tile([P, P], bf16, name="cm128_sb")
            mrow_sb = persist.tile([1, t], bf16, name="mrow_sb")
            dvalrow_sb = persist.tile([1, t], f32, name="dvalrow_sb")
            dvalB_sb = persist.tile([P, t], bf16, name="dvalB_sb")

            # small/bulk startup loads ride the gpsimd queue so the sync
            # queue can start feeding the first qkv matmuls immediately
            nc.gpsimd.dma_start(rqT_sb, rqT_d)
            nc.gpsimd.dma_start(cm128_sb, cmask128_d)
            nc.gpsimd.dma_start(mrow_sb, mrow_d)
            nc.gpsimd.dma_start(dvalrow_sb, dvalrow_d)

            # fire a tiny throwaway AllToAll immediately: the first
            # collective pays the CC-path init cost (~100us on the
            # baseline's first ReduceScatter), which this hides under
            # the qkv phase
            with tc.tile_pool(name="dram_warm", bufs=1, space="DRAM") as dwarm:
                warm_in = dwarm.tile([n_cores * 16, 16], bf16, name="cc_warm_in")
                warm_out = dwarm.tile([n_cores * 16, 16], bf16, name="cc_warm_out")
                nc.gpsimd.collective_compute(
                    "AllToAll",
                    mybir.AluOpType.bypass,
                    replica_groups=[list(range(n_cores))],
                    ins=[warm_in.opt()],
                    outs=[warm_out.opt()],
                )

            # per-head persistent activations
            qT = [persist.tile([P, t], bf16, name=f"qT{h}") for h in range(hl)]
            kT = [persist.tile([P, t], bf16, name=f"kT{h}") for h in range(hl)]
            vT = [persist.tile([P, t], bf16, name=f"vT{h}") for h in range(hl)]
            v_nat = [
                persist.tile([P, qb_n, P], bf16, name=f"vnat{h}") for h in range(hl)
            ]

            # ---------------- phase 1: qkv + rope + v transpose ----------
            with (
                tc.tile_pool(name="wq", bufs=1) as wqpool,
                tc.tile_pool(name="cs", bufs=1) as cspool,
                tc.tile_pool(name="ph1", bufs=2) as ph1,
                tc.tile_pool(name="ps_qkv", bufs=1, space="PSUM") as ps_qkv,
                tc.tile_pool(name="ps_aux", bufs=2, space="PSUM") as ps_aux,
            ):
                cosT_sb = cspool.tile([P, t], f32, name="cosT_sb")
                ssinT_sb = cspool.tile([P, t], f32, name="ssinT_sb")
                # t-chunk 0 rope tables first (sync queue, tiny), rest on
                # the gpsimd queue behind the startup loads
                nc.sync.dma_start(cosT_sb[:, 0:tch], cosT_d[:, 0:tch])
                nc.sync.dma_start(ssinT_sb[:, 0:tch], ssinT_d[:, 0:tch])
                for tc_i in range(1, nt):
                    tsl = slice(tc_i * tch, (tc_i + 1) * tch)
                    nc.gpsimd.dma_start(cosT_sb[:, tsl], cosT_d[:, tsl])
                    nc.gpsimd.dma_start(ssinT_sb[:, tsl], ssinT_d[:, tsl])
                nc.gpsimd.dma_start(dvalB_sb, dvalB_d)

                wq_sb = wqpool.tile([P, kd, 3 * hl, P], bf16, name="wq_sb")
                wqkv_r = wqkv.rearrange("(kd p) (c j) -> p kd c j", p=P, j=P)
                xT_r = xT.rearrange("(kd p) x -> p kd x", p=P)

                for tc_i in range(nt):
                    tsl = slice(tc_i * tch, (tc_i + 1) * tch)
                    xt = ph1.tile([P, kd, tch], bf16, tag="xt")
                    for k in range(kd):
                        if tc_i == 0:
                            # interleave weight + x chunks so matmul k can
                            # start as soon as its own inputs land
                            nc.sync.dma_start(wq_sb[:, k], wqkv_r[:, k])
                        nc.sync.dma_start(xt[:, k], xT_r[:, k, tsl])
                    for c in range(3 * hl):  # q0,q1,k0,k1,v0,v1
                        ps = ps_qkv.tile([P, tch], mybir.dt.float32, tag=f"ps{c}")
                        for k in range(kd):
                            nc.tensor.matmul(
                                ps,
                                lhsT=wq_sb[:, k, c],
                                rhs=xt[:, k],
                                start=(k == 0),
                                stop=(k == kd - 1),
                            )
                        if c < 2 * hl:  # q or k: cast, rotate, rope-combine
                            dst = qT[c] if c < hl else kT[c - hl]
                            qbf = ph1.tile([P, tch], bf16, tag="qbf")
                            nc.scalar.copy(qbf, ps)
                            # rotate-half: partition shift by 64 via two
                            # SBUF->SBUF DMAs (keeps PE free)
                            shift = ph1.tile([P, tch], bf16, tag="shift")
                            nc.sync.dma_start(shift[0:64], qbf[64:128])
                            nc.sync.dma_start(shift[64:128], qbf[0:64])
                            t1 = ph1.tile([P, tch], f32, tag="t1")
                            nc.vector.tensor_tensor(
                                t1, qbf, cosT_sb[:, tsl], mybir.AluOpType.mult
                            )
                            t2 = ph1.tile([P, tch], f32, tag="t2")
                            nc.vector.tensor_tensor(
                                t2, shift, ssinT_sb[:, tsl], mybir.AluOpType.mult
                            )
                            nc.vector.tensor_tensor(
                                dst[:, tsl], t1, t2, mybir.AluOpType.add
                            )
                        else:  # v: just cast
                            nc.scalar.copy(vT[c - 2 * hl][:, tsl], ps)

                # v: [DH, T] -> natural [T-block, DH] tiles
                for h in range(hl):
                    for b in range(qb_n):
                        pst = ps_aux.tile([P, P], bf16, tag="aux")
                        nc.tensor.transpose(
                            pst, vT[h][:, b * P : (b + 1) * P], ident
                        )
                        nc.scalar.copy(v_nat[h][:, b], pst)
                # vT only feeds the masked-query blend from here on:
                # premultiply by (1 - m_q) in place
                for h in range(hl):
                    nc.vector.tensor_tensor(
                        vT[h], vT[h], dvalB_sb, mybir.AluOpType.mult
                    )

            # ---------------- phase 2: SDPA + A2A + out-proj -------------
            with (
                tc.tile_pool(name="w2", bufs=1) as w2pool,
                tc.tile_pool(name="ph2", bufs=2) as ph2,
                tc.tile_pool(name="pt", bufs=3) as ptpool,
                tc.tile_pool(name="lhsp", bufs=2) as lhsppool,
                tc.tile_pool(name="dram", bufs=1, space="DRAM") as dram,
                tc.tile_pool(name="ps_s", bufs=2, space="PSUM") as ps_s,
                tc.tile_pool(name="ps_bc", bufs=1, space="PSUM") as ps_bc,
                tc.tile_pool(name="ps_o", bufs=2, space="PSUM") as ps_o,
                tc.tile_pool(name="ps_d", bufs=1, space="PSUM") as ps_d,
                tc.tile_pool(name="ps_out", bufs=2, space="PSUM") as ps_out,
            ):
                wout_sb = w2pool.tile([P, kd, d], bf16, name="wout_sb")
                nc.gpsimd.dma_start(
                    wout_sb, wout_d.rearrange("(h p) x -> p h x", p=P)
                )

                a2a_in = [
                    dram.tile([2 * hl * n_cores * P // 2, rows_per_rank], bf16,
                              name=f"a2a_in{g}")
                    for g in range(n_quads)
                ]
                a2a_out = [
                    dram.tile([2 * hl * n_cores * P // 2, rows_per_rank], bf16,
                              name=f"a2a_out{g}")
                    for g in range(n_quads)
                ]

                lhs_tiles = {}

                def emit_sdpa_blocks(g, h):
                    """PE block stream for one (quad, head): scores, exp,
                    PV + denominator accumulate.  Returns psum handles."""
                    nsk = (g + 1) * qb_per_quad
                    pso = ps_o.tile([P, qw], mybir.dt.float32, tag="pso")
                    psd = ps_d.tile([1, qw], mybir.dt.float32, tag="psd")

                    def emit_score(sk):
                        br = sk - g * qb_per_quad  # >=0 in diag region
                        lo = br * P if br >= 0 else 0
                        psT = ps_s.tile([P, qw], mybir.dt.float32, tag="scT")
                        nc.tensor.matmul(
                            psT[:, lo:],
                            lhsT=kT[h][:, sk * P : (sk + 1) * P],
                            rhs=qT[h][:, g * qw + lo : (g + 1) * qw],
                            start=True,
                            stop=True,
                        )
                        pT = ptpool.tile([P, qw], bf16, tag="pT")
                        nc.scalar.activation(
                            pT[:, lo:],
                            psT[:, lo:],
                            mybir.ActivationFunctionType.Exp,
                            scale=float(scale),
                            bias=rqT_sb[:, sk : sk + 1],
                        )
                        if br >= 0:
                            nc.vector.tensor_tensor(
                                pT[:, lo : lo + P],
                                pT[:, lo : lo + P],
                                cm128_sb,
                                mybir.AluOpType.mult,
                            )
                        return pT, lo

                    def emit_pv(sk, pT, lo):
                        nc.tensor.matmul(
                            pso[:, lo:],
                            lhsT=v_nat[h][:, sk],
                            rhs=pT[:, lo:],
                            start=(sk == 0),
                            stop=(sk == nsk - 1),
                        )
                        nc.tensor.matmul(
                            psd[:, lo:],
                            lhsT=ones_col,
                            rhs=pT[:, lo:],
                            start=(sk == 0),
                            stop=(sk == nsk - 1),
                        )

                    LA = 2
                    stage = {}
                    for sk in range(nsk):
                        stage[sk] = emit_score(sk)
                        if sk - LA >= 0:
                            emit_pv(sk - LA, *stage.pop(sk - LA))
                    for sk in range(max(0, nsk - LA), nsk):
                        emit_pv(sk, *stage.pop(sk))

                    # evacuate unnormalized oT on ACT right away so the
                    # pso psum bank frees a full sequence early
                    oraw = ph2.tile([P, qw], bf16, tag=f"oraw{h}")
                    nc.scalar.copy(oraw, pso)

                    # denominator chain on DVE (off the PE critical path):
                    # den' = den + (1-m_q); inv = 1/den'; inv *= m_q
                    gsl = slice(g * qw, (g + 1) * qw)
                    dsafe = ph2.tile([1, qw], mybir.dt.float32, tag="dsafe")
                    nc.vector.tensor_tensor(
                        dsafe, psd, dvalrow_sb[:, gsl], mybir.AluOpType.add
                    )
                    rinv = ph2.tile([1, qw], mybir.dt.float32, tag="rinv")
                    nc.vector.reciprocal(rinv, dsafe)
                    inv = ph2.tile([1, qw], bf16, tag=f"inv{h}")
                    nc.vector.tensor_tensor(
                        inv, rinv, mrow_sb[:, gsl], mybir.AluOpType.mult
                    )
                    return oraw, inv

                def emit_flush(g, h, oraw, inv):
                    """Broadcast inv to 128 partitions (PE), evacuate
                    normalized oT, stage the A2A slab; on h==1 trigger the
                    quad's collective + result load."""
                    gsl = slice(g * qw, (g + 1) * qw)
                    bc_ps = ps_bc.tile([P, qw], mybir.dt.float32, tag="bc")
                    nc.tensor.matmul(
                        bc_ps, lhsT=ones_row, rhs=inv, start=True, stop=True
                    )
                    bc_sb = ph2.tile([P, qw], bf16, tag="bc_sb")
                    nc.scalar.copy(bc_sb, bc_ps)
                    oTq = ph2.tile([P, qw], bf16, tag=f"oTq{h}")
                    nc.vector.tensor_tensor(
                        oTq, oraw, bc_sb, mybir.AluOpType.mult
                    )
                    nc.vector.tensor_tensor(
                        oTq, oTq, vT[h][:, gsl], mybir.AluOpType.add
                    )
                    a2a_in_v = a2a_in[g].rearrange(
                        "(j e p) c -> e p j c", j=n_cores, e=hl
                    )
                    nc.sync.dma_start(
                        a2a_in_v[h],
                        oTq.rearrange("p (j c) -> p j c", j=n_cores),
                    )
                    if h == hl - 1:
                        nc.gpsimd.collective_compute(
                            "AllToAll",
                            mybir.AluOpType.bypass,
                            replica_groups=[list(range(n_cores))],
                            ins=[a2a_in[g].opt()],
                            outs=[a2a_out[g].opt()],
                        )
                        p, half = g // 2, g % 2
                        if half == 0:
                            lhs_tiles[p] = lhsppool.tile(
                                [P, kd, P], bf16, tag="lhsP", name="lhsP"
                            )
                        nc.gpsimd.dma_start(
                            lhs_tiles[p][
                                :, :, half * rows_per_rank : (half + 1) * rows_per_rank
                            ],
                            a2a_out[g].rearrange(
                                "(j e p) c -> p (j e) c", j=n_cores, e=hl
                            ),
                        )

                def emit_outproj(p):
                    """Full out-projection for my 128 rows of quad pair p."""
                    lhsP = lhs_tiles.pop(p)
                    partial = ph2.tile([P, d], bf16, tag="partial")
                    for ntile in range(d // 512):
                        nsl = slice(ntile * 512, (ntile + 1) * 512)
                        pso2 = ps_out.tile([P, 512], mybir.dt.float32, tag="outps")
                        for j in range(kd):
                            nc.tensor.matmul(
                                pso2,
                                lhsT=lhsP[:, j],
                                rhs=wout_sb[:, j, nsl],
                                start=(j == 0),
                                stop=(j == kd - 1),
                            )
                        nc.scalar.copy(partial[:, nsl], pso2)
                    nc.sync.dma_start(out_d[p * P : (p + 1) * P, :], partial)

                # main loop: flush of sequence i lags behind the block
                # stream of sequence i+1 so PE never waits on DVE/A2A;
                # outproj(p) is held for two extra flush slots after its
                # second A2A triggers so the collective can complete
                pending = None
                outproj_queue = []  # entries (pair, flush_count_at_append)
                n_flushed = 0
                for g in range(n_quads):
                    for h in range(hl):
                        handles = emit_sdpa_blocks(g, h)
                        if pending is not None:
                            emit_flush(*pending)
                            n_flushed += 1
                            if outproj_queue and n_flushed >= outproj_queue[0][1] + 2:
                                emit_outproj(outproj_queue.pop(0)[0])
                            pg, phh = pending[0], pending[1]
                            if phh == hl - 1 and pg % 2 == 1:
                                outproj_queue.append((pg // 2, n_flushed))
                        pending = (g, h, *handles)
                emit_flush(*pending)
                while outproj_queue:
                    emit_outproj(outproj_queue.pop(0)[0])
                emit_outproj(n_pairs - 1)

    nc.compile()
    return nc


def prepare_in_maps(x, W_qkv, W_out, cos, sin, mask, n_cores=N_CORES, hl=H // N_CORES):
    """Host-side sharding. Returns list of per-core input dicts."""
    t, d = x.shape
    x = np.asarray(x, dtype=BF16)
    W_qkv = np.asarray(W_qkv, dtype=BF16)
    W_out = np.asarray(W_out, dtype=BF16)
    cos = np.asarray(cos, dtype=np.float32)
    sin = np.asarray(sin, dtype=np.float32)
    m = np.asarray(mask, dtype=bool)

    xT = np.ascontiguousarray(x.T)
    cosT = np.ascontiguousarray(cos.T)
    sign = np.where(np.arange(DH) < DH // 2, -1.0, 1.0).astype(np.float32)
    ssinT = np.ascontiguousarray(sin.T * sign[:, None])

    mf = m.astype(np.float32)
    rqT = np.ascontiguousarray(
        np.where(mf, np.float32(0.0), np.float32(-1e9)).reshape(-1, DH).T
    )
    dvalB = np.ascontiguousarray(
        np.broadcast_to((1.0 - mf).astype(BF16)[None, :], (DH, t))
    )
    mrow = np.ascontiguousarray(mf.astype(BF16).reshape(1, t))
    dvalrow = np.ascontiguousarray((1.0 - mf).reshape(1, t))
    cmask128 = (np.arange(DH)[None, :] >= np.arange(DH)[:, None]).astype(BF16)

    n_heads = W_qkv.shape[1] // 3 // DH
    in_maps = []
    for c in range(n_cores):
        hs = [c * hl + i for i in range(hl)]
        cols = [W_qkv[:, (s * n_heads + h) * DH : (s * n_heads + h) * DH + DH]
                for s in range(3) for h in hs]
        wqkv_c = np.ascontiguousarray(np.concatenate(cols, axis=1))
        in_maps.append(
            {
                "xT": xT,
                "wqkv": wqkv_c,
                "wout": W_out,
                "cosT": cosT,
                "ssinT": ssinT,
                "rqT": rqT,
                "dvalB": dvalB,
                "mrow": mrow,
                "dvalrow": dvalrow,
                "cmask128": cmask128,
            }
        )
    return in_maps


_CACHED_NC = None


def assemble(results, t=T, d=D, n_cores=N_CORES):
    """Reassemble per-core A2A row shards into the full output.
    Core r's out row g*64 + i is global row 512*g + 64*r + i."""
    qw = 512
    rows = qw // n_cores  # 64
    out = np.empty((t, d), dtype=BF16)
    for r in range(n_cores):
        oc = np.asarray(results[r]["out"])
        if oc.dtype != BF16:
            oc = oc.view(BF16)
        for g in range(t // qw):
            out[qw * g + rows * r : qw * g + rows * (r + 1)] = oc[
                rows * g : rows * (g + 1)
            ]
    return out


def kernel(x, W_qkv, W_out, cos, sin, mask):
    """Full inputs in, full output out. Shards across 8 NeuronCores."""
    global _CACHED_NC
    from concourse import bass_utils

    if _CACHED_NC is None:
        _CACHED_NC = build_nc()
    nc = _CACHED_NC

    in_maps = prepare_in_maps(x, W_qkv, W_out, cos, sin, mask)
    res = bass_utils.run_bass_kernel_spmd(
        nc, in_maps, core_ids=list(range(N_CORES))
    )
    return assemble(res.results)


# revision 16
# speedup vs baseline: 1.2775x; 1.0267x over previous
"""Distributed Trainium2 attention kernel (8 NeuronCores, head tensor-parallel).

Reference semantics (T=4096, D=2048, H=16, DH=128):
  qkv = bf16(x @ W_qkv); q,k,v per head; RoPE(split-half) on q,k;
  mask = ((m_q & m_k) | eye) & causal; softmax(q k^T / sqrt(DH) masked);
  out = bf16((probs @ v) @ W_out)

Sharding: head tensor-parallel for qkv+SDPA (core c owns heads 2c, 2c+1),
then an AllToAll redistributes the small per-head attention outputs o so
that each core owns 64 output ROWS per 512-query quad (512 rows total)
and computes the full out-projection locally against a replicated W_out.
This moves 8x fewer bytes than reduce-scattering output partials.

Device-side layout choices:
  - x passed as xT [D, T] so the D contraction dim is the partition dim.
  - q,k computed weight-stationary -> born transposed [DH, T]; v
    transposed back to natural [T, DH] via PE (PV lhsT layout).
  - RoPE: partition-rotate by 64 via two SBUF->SBUF DMAs, sign folded
    into a host-precomputed ssinT table; combine on DVE.
  - SDPA in transposed-scores form: scoresT[k, q] tiles over 512-query
    quads; exp (no max-subtraction; scores are O(5)) evacuates the
    scores psum straight into the PV rhs; key padding mask folded into
    the exp bias (per-k = per-partition); within-block causal via one
    precomputed 0/1 [128,128] multiply; softmax denominators via a
    ones-column matmul.
  - Normalization fused into the oT evacuation: inv = m_q/(den+(1-m_q))
    computed on one partition, broadcast to all 128 partitions with a
    K=1 ones-outer-product matmul on PE, then oT = pso*bc + vT*(1-m_q)
    (vT premultiplied by (1-m_q) once). Masked queries thereby attend
    only to themselves; out-projection needs no per-tile scaling.
  - bc matmuls and evacuations lag one head-sequence behind the PE
    block stream so the PE never waits on the DVE denominator chain.
  - A2A per quad: [2048, 64] bf16 (shard j = my heads' oT columns for
    rank j's 64 rows of this quad). Collective-dependent loads ride the
    GpSimd DMA queue so they cannot head-of-line-block the Sync queue.
"""

import os
import sys

import numpy as np

sys.path.insert(0, "/opt/trn_rl_repo")

import ml_dtypes

BF16 = ml_dtypes.bfloat16

# problem constants (hardcoded per harness contract)
T, D, H, DH = 4096, 2048, 16, 128
N_CORES = 8
ROPE_BASE = 10000.0


def build_nc(
    t=T,
    d=D,
    n_cores=N_CORES,
    hl=H // N_CORES,  # heads per core
    tch=512,  # qkv t-chunk
):
    import concourse.bass as bass
    import concourse.mybir as mybir
    import concourse.tile as tile
    from concourse import bacc
    from concourse.masks import make_identity

    f32 = mybir.dt.float32
    bf16 = mybir.dt.bfloat16

    P = 128
    kd = d // P  # contraction chunks for qkv
    qb_n = t // P  # q-blocks of 128 rows
    nt = t // tch  # t-chunks in qkv phase
    qw = 512  # queries per quad
    n_quads = t // qw
    qb_per_quad = qw // P  # 4
    rows_per_rank = qw // n_cores  # 64 rows each rank owns per quad
    n_pairs = n_quads // 2
    t_out = t // n_cores  # output rows per core
    scale = 1.0 / np.sqrt(DH)

    nc = bacc.Bacc(
        "TRN2", target_bir_lowering=False, debug=False, num_devices=n_cores
    )

    xT = nc.dram_tensor("xT", [d, t], bf16, kind="ExternalInput").ap()
    wqkv = nc.dram_tensor("wqkv", [d, 3 * hl * P], bf16, kind="ExternalInput").ap()
    wout_d = nc.dram_tensor("wout", [d, d], bf16, kind="ExternalInput").ap()
    cosT_d = nc.dram_tensor("cosT", [P, t], f32, kind="ExternalInput").ap()
    ssinT_d = nc.dram_tensor("ssinT", [P, t], f32, kind="ExternalInput").ap()
    # rqT[p, qb] = 0 if mask[qb*128+p] else -1e9 (folded into exp bias)
    rqT_d = nc.dram_tensor("rqT", [P, qb_n], f32, kind="ExternalInput").ap()
    # dvalB[p, q] = 1 - mask[q], broadcast to all partitions
    dvalB_d = nc.dram_tensor("dvalB", [P, t], bf16, kind="ExternalInput").ap()
    # mrow[0, q] = mask[q] ; dvalrow[0, q] = 1 - mask[q]
    mrow_d = nc.dram_tensor("mrow", [1, t], bf16, kind="ExternalInput").ap()
    dvalrow_d = nc.dram_tensor("dvalrow", [1, t], f32, kind="ExternalInput").ap()
    # cmask128[p, j] = 1 if j >= p else 0 (within-block causal, T-orientation)
    cmask128_d = nc.dram_tensor("cmask128", [P, P], bf16, kind="ExternalInput").ap()
    out_d = nc.dram_tensor("out", [t_out, d], bf16, kind="ExternalOutput").ap()

    with tile.TileContext(nc) as tc:
        with tc.tile_pool(name="persist", bufs=1) as persist:
            ident = persist.tile([P, P], bf16, name="ident")
            make_identity(nc, ident)
            ones_col = persist.tile([P, 1], bf16, name="ones_col")
            nc.vector.memset(ones_col, 1.0)
            ones_row = persist.tile([1, P], bf16, name="ones_row")
            nc.vector.memset(ones_row, 1.0)
            rqT_sb = persist.tile([P, qb_n], f32, name="rqT_sb")
            cm128_sb = persist.tile([P, P], bf16, name="cm128_sb")
            mrow_sb = persist.tile([1, t], bf16, name="mrow_sb")
            dvalrow_sb = persist.tile([1, t], f32, name="dvalrow_sb")
            dvalB_sb = persist.tile([P, t], bf16, name="dvalB_sb")

            # small/bulk startup loads ride the gpsimd queue so the sync
            # queue can start feeding the first qkv matmuls immediately
            nc.gpsimd.dma_start(rqT_sb, rqT_d)
            nc.gpsimd.dma_start(cm128_sb, cmask128_d)
            nc.gpsimd.dma_start(mrow_sb, mrow_d)
            nc.gpsimd.dma_start(dvalrow_sb, dvalrow_d)

            # fire a tiny throwaway AllToAll immediately: the first
            # collective pays the CC-path init cost (~100us on the
            # baseline's first ReduceScatter), which this hides under
            # the qkv phase
            with tc.tile_pool(name="dram_warm", bufs=1, space="DRAM") as dwarm:
                warm_in = dwarm.tile([n_cores * 16, 16], bf16, name="cc_warm_in")
                warm_out = dwarm.tile([n_cores * 16, 16], bf16, name="cc_warm_out")
                nc.gpsimd.collective_compute(
                    "AllToAll",
                    mybir.AluOpType.bypass,
                    replica_groups=[list(range(n_cores))],
                    ins=[warm_in.opt()],
                    outs=[warm_out.opt()],
                )

            # per-head persistent activations
            qT = [persist.tile([P, t], bf16, name=f"qT{h}") for h in range(hl)]
            kT = [persist.tile([P, t], bf16, name=f"kT{h}") for h in range(hl)]
            vT = [persist.tile([P, t], bf16, name=f"vT{h}") for h in range(hl)]
            v_nat = [
                persist.tile([P, qb_n, P], bf16, name=f"vnat{h}") for h in range(hl)
            ]

            # ---------------- phase 1: qkv + rope + v transpose ----------
            with (
                tc.tile_pool(name="wq", bufs=1) as wqpool,
                tc.tile_pool(name="cs", bufs=1) as cspool,
                tc.tile_pool(name="ph1", bufs=2) as ph1,
                tc.tile_pool(name="ps_qkv", bufs=1, space="PSUM") as ps_qkv,
                tc.tile_pool(name="ps_aux", bufs=2, space="PSUM") as ps_aux,
            ):
                cosT_sb = cspool.tile([P, t], f32, name="cosT_sb")
                ssinT_sb = cspool.tile([P, t], f32, name="ssinT_sb")
                # t-chunk 0 rope tables first (sync queue, tiny), rest on
                # the gpsimd queue behind the startup loads
                nc.sync.dma_start(cosT_sb[:, 0:tch], cosT_d[:, 0:tch])
                nc.sync.dma_start(ssinT_sb[:, 0:tch], ssinT_d[:, 0:tch])
                for tc_i in range(1, nt):
                    tsl = slice(tc_i * tch, (tc_i + 1) * tch)
                    nc.gpsimd.dma_start(cosT_sb[:, tsl], cosT_d[:, tsl])
                    nc.gpsimd.dma_start(ssinT_sb[:, tsl], ssinT_d[:, tsl])
                nc.gpsimd.dma_start(dvalB_sb, dvalB_d)

                # weights channel-major so channel 0's 16 k-chunks land
                # first and the very first matmul group isn't DMA-starved
                wq_sb = wqpool.tile([P, 3 * hl, kd, P], bf16, name="wq_sb")
                wqkv_r = wqkv.rearrange("(kd p) (c j) -> p c kd j", p=P, j=P)
                xT_r = xT.rearrange("(kd p) x -> p kd x", p=P)

                xts = {}

                def load_xt(tc_i):
                    xt = ph1.tile([P, kd, tch], bf16, tag="xt")
                    tsl = slice(tc_i * tch, (tc_i + 1) * tch)
                    for k in range(kd):
                        nc.sync.dma_start(xt[:, k], xT_r[:, k, tsl])
                        if tc_i == 0 and k < 4:
                            # channel 0's weights interleave with the first
                            # x chunks; remaining channels follow behind
                            nc.sync.dma_start(
                                wq_sb[:, 0, k * 4 : (k + 1) * 4],
                                wqkv_r[:, 0, k * 4 : (k + 1) * 4],
                            )
                    if tc_i == 0:
                        for c in range(1, 3 * hl):
                            nc.sync.dma_start(wq_sb[:, c], wqkv_r[:, c])
                    xts[tc_i] = xt

                pending_vt = []

                def flush_vt():
                    h, b0, b1 = pending_vt.pop(0)
                    for b in range(b0, b1):
                        pst = ps_aux.tile([P, P], bf16, tag="aux")
                        nc.tensor.transpose(
                            pst, vT[h][:, b * P : (b + 1) * P], ident
                        )
                        nc.scalar.copy(v_nat[h][:, b], pst)

                load_xt(0)
                for tc_i in range(nt):
                    tsl = slice(tc_i * tch, (tc_i + 1) * tch)
                    if tc_i + 1 < nt:
                        load_xt(tc_i + 1)  # prefetch one chunk ahead
                    xt = xts.pop(tc_i)
                    for c in range(3 * hl):  # q0,q1,k0,k1,v0,v1
                        ps = ps_qkv.tile([P, tch], mybir.dt.float32, tag=f"ps{c}")
                        for k in range(kd):
                            nc.tensor.matmul(
                                ps,
                                lhsT=wq_sb[:, c, k],
                                rhs=xt[:, k],
                                start=(k == 0),
                                stop=(k == kd - 1),
                            )
                        if pending_vt:
                            # v transposes lag one channel behind so the PE
                            # never waits on the vT psum->sbuf cast
                            flush_vt()
                        if c < 2 * hl:  # q or k: cast, rotate, rope-combine
                            dst = qT[c] if c < hl else kT[c - hl]
                            qbf = ph1.tile([P, tch], bf16, tag="qbf")
                            nc.scalar.copy(qbf, ps)
                            # rotate-half: partition shift by 64 via two
                            # SBUF->SBUF DMAs (keeps PE free; scalar queue
                            # so the sync queue keeps streaming x chunks)
                            shift = ph1.tile([P, tch], bf16, tag="shift")
                            nc.scalar.dma_start(shift[0:64], qbf[64:128])
                            nc.scalar.dma_start(shift[64:128], qbf[0:64])
                            t1 = ph1.tile([P, tch], f32, tag="t1")
                            nc.vector.tensor_tensor(
                                t1, qbf, cosT_sb[:, tsl], mybir.AluOpType.mult
                            )
                            t2 = ph1.tile([P, tch], f32, tag="t2")
                            nc.vector.tensor_tensor(
                                t2, shift, ssinT_sb[:, tsl], mybir.AluOpType.mult
                            )
                            nc.vector.tensor_tensor(
                                dst[:, tsl], t1, t2, mybir.AluOpType.add
                            )
                        else:  # v: cast; queue this chunk's transposes
                            h = c - 2 * hl
                            nc.scalar.copy(vT[h][:, tsl], ps)
                            pending_vt.append(
                                (h, tc_i * tch // P, (tc_i + 1) * tch // P)
                            )
                while pending_vt:
                    flush_vt()

                # vT only feeds the masked-query blend from here on:
                # premultiply by (1 - m_q) in place
                for h in range(hl):
                    nc.vector.tensor_tensor(
                        vT[h], vT[h], dvalB_sb, mybir.AluOpType.mult
                    )

            # ---------------- phase 2: SDPA + A2A + out-proj -------------
            with (
                tc.tile_pool(name="w2", bufs=1) as w2pool,
                tc.tile_pool(name="ph2", bufs=2) as ph2,
                tc.tile_pool(name="pt", bufs=4) as ptpool,
                tc.tile_pool(name="lhsp", bufs=2) as lhsppool,
                tc.tile_pool(name="dram", bufs=1, space="DRAM") as dram,
                tc.tile_pool(name="ps_s", bufs=3, space="PSUM") as ps_s,
                tc.tile_pool(name="ps_o", bufs=2, space="PSUM") as ps_o,
                tc.tile_pool(name="ps_d", bufs=1, space="PSUM") as ps_d,
                tc.tile_pool(name="ps_out", bufs=2, space="PSUM") as ps_out,
            ):
                wout_sb = w2pool.tile([P, kd, d], bf16, name="wout_sb")
                nc.gpsimd.dma_start(
                    wout_sb, wout_d.rearrange("(h p) x -> p h x", p=P)
                )

                a2a_in = [
                    dram.tile([2 * hl * n_cores * P // 2, rows_per_rank], bf16,
                              name=f"a2a_in{g}")
                    for g in range(n_quads)
                ]
                a2a_out = [
                    dram.tile([2 * hl * n_cores * P // 2, rows_per_rank], bf16,
                              name=f"a2a_out{g}")
                    for g in range(n_quads)
                ]

                lhs_tiles = {}

                def emit_sdpa_blocks(g, h):
                    """PE block stream for one (quad, head): scores, exp,
                    PV + denominator accumulate.  Returns psum handles."""
                    nsk = (g + 1) * qb_per_quad
                    pso = ps_o.tile([P, qw], mybir.dt.float32, tag="pso")
                    psd = ps_d.tile([1, qw], mybir.dt.float32, tag="psd")

                    def emit_score(sk):
                        br = sk - g * qb_per_quad  # >=0 in diag region
                        lo = br * P if br >= 0 else 0
                        psT = ps_s.tile([P, qw], mybir.dt.float32, tag="scT")
                        nc.tensor.matmul(
                            psT[:, lo:],
                            lhsT=kT[h][:, sk * P : (sk + 1) * P],
                            rhs=qT[h][:, g * qw + lo : (g + 1) * qw],
                            start=True,
                            stop=True,
                        )
                        pT = ptpool.tile([P, qw], bf16, tag="pT")
                        nc.scalar.activation(
                            pT[:, lo:],
                            psT[:, lo:],
                            mybir.ActivationFunctionType.Exp,
                            scale=float(scale),
                            bias=rqT_sb[:, sk : sk + 1],
                        )
                        if br >= 0:
                            nc.vector.tensor_tensor(
                                pT[:, lo : lo + P],
                                pT[:, lo : lo + P],
                                cm128_sb,
                                mybir.AluOpType.mult,
                            )
                        return pT, lo

                    def emit_pv(sk, pT, lo):
                        nc.tensor.matmul(
                            pso[:, lo:],
                            lhsT=v_nat[h][:, sk],
                            rhs=pT[:, lo:],
                            start=(sk == 0),
                            stop=(sk == nsk - 1),
                        )
                        nc.tensor.matmul(
                            psd[:, lo:],
                            lhsT=ones_col,
                            rhs=pT[:, lo:],
                            start=(sk == 0),
                            stop=(sk == nsk - 1),
                        )

                    LA = 2
                    stage = {}
                    for sk in range(nsk):
                        stage[sk] = emit_score(sk)
                        if sk - LA >= 0:
                            emit_pv(sk - LA, *stage.pop(sk - LA))
                    for sk in range(max(0, nsk - LA), nsk):
                        emit_pv(sk, *stage.pop(sk))

                    # evacuate unnormalized oT (DVE; ACT does exp only in
                    # this phase) right away so the pso psum bank frees a
                    # full sequence early
                    oraw = ph2.tile([P, qw], bf16, tag=f"oraw{h}")
                    nc.vector.tensor_copy(oraw, pso)

                    # denominator chain on DVE (off the PE critical path):
                    # den' = den + (1-m_q); inv = 1/den'; inv *= m_q
                    gsl = slice(g * qw, (g + 1) * qw)
                    dsafe = ph2.tile([1, qw], mybir.dt.float32, tag="dsafe")
                    nc.vector.tensor_tensor(
                        dsafe, psd, dvalrow_sb[:, gsl], mybir.AluOpType.add
                    )
                    rinv = ph2.tile([1, qw], mybir.dt.float32, tag="rinv")
                    nc.vector.reciprocal(rinv, dsafe)
                    inv = ph2.tile([1, qw], bf16, tag=f"inv{h}")
                    nc.vector.tensor_tensor(
                        inv, rinv, mrow_sb[:, gsl], mybir.AluOpType.mult
                    )
                    return oraw, inv

                def emit_flush(g, h, oraw, inv):
                    """Broadcast inv to 128 partitions (PE), evacuate
                    normalized oT, stage the A2A slab; on h==1 trigger the
                    quad's collective + result load."""
                    gsl = slice(g * qw, (g + 1) * qw)
                    bc_ps = ps_s.tile([P, qw], mybir.dt.float32, tag="scT")
                    nc.tensor.matmul(
                        bc_ps, lhsT=ones_row, rhs=inv, start=True, stop=True
                    )
                    bc_sb = ph2.tile([P, qw], bf16, tag="bc_sb")
                    nc.vector.tensor_copy(bc_sb, bc_ps)
                    oTq = ph2.tile([P, qw], bf16, tag=f"oTq{h}")
                    nc.vector.tensor_tensor(
                        oTq, oraw, bc_sb, mybir.AluOpType.mult
                    )
                    nc.vector.tensor_tensor(
                        oTq, oTq, vT[h][:, gsl], mybir.AluOpType.add
                    )
                    a2a_in_v = a2a_in[g].rearrange(
                        "(j e p) c -> e p j c", j=n_cores, e=hl
                    )
                    nc.sync.dma_start(
                        a2a_in_v[h],
                        oTq.rearrange("p (j c) -> p j c", j=n_cores),
                    )
                    if h == hl - 1:
                        nc.gpsimd.collective_compute(
                            "AllToAll",
                            mybir.AluOpType.bypass,
                            replica_groups=[list(range(n_cores))],
                            ins=[a2a_in[g].opt()],
                            outs=[a2a_out[g].opt()],
                        )
                        p, half = g // 2, g % 2
                        if half == 0:
                            lhs_tiles[p] = lhsppool.tile(
                                [P, kd, P], bf16, tag="lhsP", name="lhsP"
                            )
                        nc.gpsimd.dma_start(
                            lhs_tiles[p][
                                :, :, half * rows_per_rank : (half + 1) * rows_per_rank
                            ],
                            a2a_out[g].rearrange(
                                "(j e p) c -> p (j e) c", j=n_cores, e=hl
                            ),
                        )

                def emit_outproj(p):
                    """Full out-projection for my 128 rows of quad pair p."""
                    lhsP = lhs_tiles.pop(p)
                    partial = ph2.tile([P, d], bf16, tag="partial")
                    for ntile in range(d // 512):
                        nsl = slice(ntile * 512, (ntile + 1) * 512)
                        pso2 = ps_out.tile([P, 512], mybir.dt.float32, tag="outps")
                        for j in range(kd):
                            nc.tensor.matmul(
                                pso2,
                                lhsT=lhsP[:, j],
                                rhs=wout_sb[:, j, nsl],
                                start=(j == 0),
                                stop=(j == kd - 1),
                            )
                        nc.vector.tensor_copy(partial[:, nsl], pso2)
                    nc.sync.dma_start(out_d[p * P : (p + 1) * P, :], partial)

                # main loop: flush of sequence i lags behind the block
                # stream of sequence i+1 so PE never waits on DVE/A2A;
                # outproj(p) is held for two extra flush slots after its
                # second A2A triggers so the collective can complete
                pending = None
                outproj_queue = []  # entries (pair, flush_count_at_append)
                n_flushed = 0
                for g in range(n_quads):
                    for h in range(hl):
                        handles = emit_sdpa_blocks(g, h)
                        if pending is not None:
                            emit_flush(*pending)
                            n_flushed += 1
                            if outproj_queue and n_flushed >= outproj_queue[0][1] + 2:
                                emit_outproj(outproj_queue.pop(0)[0])
                            pg, phh = pending[0], pending[1]
                            if phh == hl - 1 and pg % 2 == 1:
                                outproj_queue.append((pg // 2, n_flushed))
                        pending = (g, h, *handles)
                emit_flush(*pending)
                while outproj_queue:
                    emit_outproj(outproj_queue.pop(0)[0])
                emit_outproj(n_pairs - 1)

    nc.compile()
    return nc


def prepare_in_maps(x, W_qkv, W_out, cos, sin, mask, n_cores=N_CORES, hl=H // N_CORES):
    """Host-side sharding. Returns list of per-core input dicts."""
    t, d = x.shape
    x = np.asarray(x, dtype=BF16)
    W_qkv = np.asarray(W_qkv, dtype=BF16)
    W_out = np.asarray(W_out, dtype=BF16)
    cos = np.asarray(cos, dtype=np.float32)
    sin = np.asarray(sin, dtype=np.float32)
    m = np.asarray(mask, dtype=bool)

    xT = np.ascontiguousarray(x.T)
    cosT = np.ascontiguousarray(cos.T)
    sign = np.where(np.arange(DH) < DH // 2, -1.0, 1.0).astype(np.float32)
    ssinT = np.ascontiguousarray(sin.T * sign[:, None])

    mf = m.astype(np.float32)
    rqT = np.ascontiguousarray(
        np.where(mf, np.float32(0.0), np.float32(-1e9)).reshape(-1, DH).T
    )
    dvalB = np.ascontiguousarray(
        np.broadcast_to((1.0 - mf).astype(BF16)[None, :], (DH, t))
    )
    mrow = np.ascontiguousarray(mf.astype(BF16).reshape(1, t))
    dvalrow = np.ascontiguousarray((1.0 - mf).reshape(1, t))
    cmask128 = (np.arange(DH)[None, :] >= np.arange(DH)[:, None]).astype(BF16)

    n_heads = W_qkv.shape[1] // 3 // DH
    in_maps = []
    for c in range(n_cores):
        hs = [c * hl + i for i in range(hl)]
        cols = [W_qkv[:, (s * n_heads + h) * DH : (s * n_heads + h) * DH + DH]
                for s in range(3) for h in hs]
        wqkv_c = np.ascontiguousarray(np.concatenate(cols, axis=1))
        in_maps.append(
            {
                "xT": xT,
                "wqkv": wqkv_c,
                "wout": W_out,
                "cosT": cosT,
                "ssinT": ssinT,
                "rqT": rqT,
                "dvalB": dvalB,
                "mrow": mrow,
                "dvalrow": dvalrow,
                "cmask128": cmask128,
            }
        )
    return in_maps


_CACHED_NC = None


def assemble(results, t=T, d=D, n_cores=N_CORES):
    """Reassemble per-core A2A row shards into the full output.
    Core r's out row g*64 + i is global row 512*g + 64*r + i."""
    qw = 512
    rows = qw // n_cores  # 64
    out = np.empty((t, d), dtype=BF16)
    for r in range(n_cores):
        oc = np.asarray(results[r]["out"])
        if oc.dtype != BF16:
            oc = oc.view(BF16)
        for g in range(t // qw):
            out[qw * g + rows * r : qw * g + rows * (r + 1)] = oc[
                rows * g : rows * (g + 1)
            ]
    return out


def kernel(x, W_qkv, W_out, cos, sin, mask):
    """Full inputs in, full output out. Shards across 8 NeuronCores."""
    global _CACHED_NC
    from concourse import bass_utils

    if _CACHED_NC is None:
        _CACHED_NC = build_nc()
    nc = _CACHED_NC

    in_maps = prepare_in_maps(x, W_qkv, W_out, cos, sin, mask)
    res = bass_utils.run_bass_kernel_spmd(
        nc, in_maps, core_ids=list(range(N_CORES))
    )
    return assemble(res.results)
